# revision 17
# baseline (speedup 1.0000x reference)
"""GroupedAttention Trainium2 kernel (8 NeuronCores, SPMD, no collectives).

Problem: x[2,2048,1024] -> grouped qkv (G=8 block-diag) -> 16-head attention
-> grouped proj (G=8 block-diag) + bias.

Sharding: core c owns heads (2c, 2c+1) and proj group c. The proj group c
consumes exactly the attention outputs of heads 2c/2c+1 and produces output
channels [128c, 128c+128) -- each core computes an independent channel
slice of the final output; outputs are concatenated on the host.

Engine plan (per core):
- PE: q/k/v projections (f32r / fp16), scores S=K^T.Q (f32r, 512-col moving),
  AV with stationary P-tile [m, n-block] and moving V|ones [m, 65] fp16
  accumulating out[n, 65] over m-tiles (the 65th column collects the softmax
  denominator), proj (fp16) with bias folded in via a K=1 ones-row matmul.
- ACT: part of the exp tiles (table Exp, scale folded in), psum->sbuf copies.
- DVE: the other exp tiles via Schraudolph bit-trick exp
  (fp16 = bitcast(int16(x*SCALE*log2e*1024 + 15315.26))), v copies,
  per-partition softmax normalization.
- SP (sync): all DMA issue - input loads, XBAR dma-transposes of the
  normalized attention tiles [n,128]->[128,n], output stores.
"""

import numpy as np
from contextlib import ExitStack

import concourse.bass as bass
import concourse.tile as tile
from concourse import bacc, mybir
from concourse.bass_utils import run_bass_kernel_spmd

F32 = mybir.dt.float32
F32R = mybir.dt.float32r
F16 = mybir.dt.float16
I16 = mybir.dt.int16
EXP = mybir.ActivationFunctionType.Exp
COPY = mybir.ActivationFunctionType.Copy

B = 2
N = 2048
C = 1024
H = 16
G = 8
D = 64          # head dim
BN = B * N      # 4096
W = 512         # attention n-window per round
NB = N // W     # rounds per batch = 4
MT = N // 128   # m-tiles per batch = 16
SCALE = D ** -0.5
LOG2E = 1.4426950408889634
# Schraudolph fp16 exp: fp16 = bitcast(int16(x*SCL1 + SCL2))
SCL1 = SCALE * LOG2E * 1024.0
SCL2 = 15315.26
# of the 16 m-tiles per window, this many run exp on ACT (rest on DVE)
ACT_TILES = 16

_CACHE = {}


def _r(ap):
    return ap if ap.dtype == F32R else ap.bitcast(F32R)


def _build_nc():
    nc = bacc.Bacc("TRN2", target_bir_lowering=False, debug=False, num_devices=8)

    xq = nc.dram_tensor("xq", [128, BN], F32, kind="ExternalInput").ap()
    xk = nc.dram_tensor("xk", [128, BN], F32, kind="ExternalInput").ap()
    xv = nc.dram_tensor("xv", [128, BN], F16, kind="ExternalInput").ap()
    wq = nc.dram_tensor("wq", [128, 128], F32, kind="ExternalInput").ap()
    wk = nc.dram_tensor("wk", [128, 128], F32, kind="ExternalInput").ap()
    wv = nc.dram_tensor("wv", [128, 128], F16, kind="ExternalInput").ap()
    wp = nc.dram_tensor("wp", [128, 128], F16, kind="ExternalInput").ap()
    brow = nc.dram_tensor("brow", [1, 128], F16, kind="ExternalInput").ap()
    y = nc.dram_tensor("y", [B, N, 128], F32, kind="ExternalOutput").ap()

    with ExitStack() as ctx:
        tc = ctx.enter_context(tile.TileContext(nc))
        nc_ = tc.nc

        persist = ctx.enter_context(tc.tile_pool(name="persist", bufs=1))

        # ---- weights / constants ----
        wk_t = persist.tile([128, 128], F32R, tag="wk")
        nc_.gpsimd.dma_start(out=wk_t, in_=wk)
        wq_t = persist.tile([128, 128], F32R, tag="wq")
        nc_.gpsimd.dma_start(out=wq_t, in_=wq)
        wv_t = persist.tile([128, 128], F16, tag="wv")
        nc_.sync.dma_start(out=wv_t, in_=wv)
        wp_t = persist.tile([128, 128], F16, tag="wp")
        nc_.sync.dma_start(out=wp_t, in_=wp)
        brow_t = persist.tile([1, 128], F16, tag="brow")
        nc_.sync.dma_start(out=brow_t, in_=brow)
        ones1 = persist.tile([1, 128], F16, tag="ones1")
        nc_.gpsimd.memset(ones1, 1.0)
        onesm = persist.tile([128, 1], F16, tag="onesm")
        nc_.gpsimd.memset(onesm, 1.0)
        ebias = persist.tile([128, 1], F32, tag="ebias")
        nc_.gpsimd.memset(ebias, EXP_BIAS)
        zero8 = persist.tile([128, 2, 4, 1], F32, tag="zero8")
        nc_.gpsimd.memset(zero8, 0.0)


        # ---- x slices (channel-major) ----
        xq_t = persist.tile([128, BN], F32R, tag="xq")
        xk_t = persist.tile([128, BN], F32R, tag="xk")
        xv_t = persist.tile([128, BN], F16, tag="xv")

        # persistent activations
        qT = [persist.tile([128, N], F32R, tag=f"qT{b}", name=f"qT{b}")
              for b in range(B)]   # rows 0:64 h0, 64:128 h1 (pre-scaled? no)
        kT = [persist.tile([128, N], F32R, tag=f"kT{b}", name=f"kT{b}")
              for b in range(B)]
        # vaug[b]: [128(m), MT, 2(h), 64] fp16
        vaug = [persist.tile([128, MT, 2, 64], F16, tag=f"vaug{b}", name=f"vaug{b}")
                for b in range(B)]
        # transposed, normalized attention outputs per b: [128(2h*d), N] fp16
        attT = [persist.tile([128, N], F16, tag=f"attT{b}", name=f"attT{b}")
                for b in range(B)]

        # ---- load x: per-batch chunks ordered k, v, q so windows start early
        for b in range(B):
            s = slice(b * N, (b + 1) * N)
            for i in range(2):
                ss = slice(b * N + i * 1024, b * N + (i + 1) * 1024)
                nc_.gpsimd.dma_start(out=xk_t[:, ss], in_=xk[:, ss])
            nc_.sync.dma_start(out=xv_t[:, s], in_=xv[:, s])
            for i in range(2):
                ss = slice(b * N + i * 1024, b * N + (i + 1) * 1024)
                nc_.gpsimd.dma_start(out=xq_t[:, ss], in_=xq[:, ss])

        # ---- phase 1: kT / vaug / qT per batch ----
        with tc.tile_pool(name="ph1", bufs=3, space="PSUM") as ph1:
            for b in range(B):
                for i in range(4):
                    sl = slice(i * 512, (i + 1) * 512)
                    s = slice(b * N + i * 512, b * N + (i + 1) * 512)
                    pk = ph1.tile([128, 512], F32, tag="qk")
                    nc_.tensor.matmul(pk, _r(wk_t), _r(xk_t[:, s]), start=True, stop=True)
                    nc_.scalar.activation(out=kT[b][:, sl], in_=pk, func=COPY)
                for mt in range(MT):
                    g = b * MT + mt
                    pv = ph1.tile([128, 128], F32, tag="v")
                    nc_.tensor.matmul(
                        pv, xv_t[:, g * 128:(g + 1) * 128], wv_t,
                        start=True, stop=True,
                    )
                    # both heads' v [128, (2,64)] -> vaug[b][:, mt]
                    nc_.vector.tensor_copy(out=vaug[b][:, mt], in_=pv)
                for i in range(4):
                    sl = slice(i * 512, (i + 1) * 512)
                    s = slice(b * N + i * 512, b * N + (i + 1) * 512)
                    pq = ph1.tile([128, 512], F32, tag="qk")
                    nc_.tensor.matmul(pq, _r(wq_t), _r(xq_t[:, s]), start=True, stop=True)
                    nc_.scalar.activation(out=qT[b][:, sl], in_=pq, func=COPY)

        # ---- phase 2: flat software-pipelined attention + proj ----
        # Iterations k = (window, m-tile); scores(k) + exp(k) are emitted at
        # k, AV(k - LAG) trails. Z rides in a single never-reset cumulative
        # PSUM bank indexed by window parity (Z_w = cum_w - cum_{w-2}), so
        # nothing has to read window state right at the window boundary:
        # normalization for window w is deferred into window w+1.
        with tc.tile_pool(name="stp", bufs=2, space="PSUM") as stp, \
             tc.tile_pool(name="avp", bufs=2, space="PSUM") as avp, \
             tc.tile_pool(name="prj", bufs=1, space="PSUM") as prj, \
             tc.tile_pool(name="ptp", bufs=5) as ptp, \
             tc.tile_pool(name="nrm", bufs=3) as nrm, \
             tc.tile_pool(name="atn", bufs=2) as atn, \
             tc.tile_pool(name="outp", bufs=2) as outp:

            LAG = 3
            NW = B * NB
            seq = [(b, nb, mt) for b in range(B) for nb in range(NB)
                   for mt in range(MT)]
            zzs = avp.tile([128, 2, 2, 4, 1], F32, tag="zzs", bufs=1,
                           name="zzs")
            avs = {}
            pts = {}
            snaps = {-2: zero8, -1: zero8}

            def emit_proj(w):
                # out[n, cout] = attT^T @ wp + ones^T @ bias
                b, nb = divmod(w, NB)
                n0 = nb * W
                pp = prj.tile([128, 4, 128], F32, tag="pp", name=f"pp{w}")
                for j in range(4):
                    nt = slice(n0 + j * 128, n0 + (j + 1) * 128)
                    nc_.tensor.matmul(pp[:, j, :], attT[b][:, nt], wp_t,
                                      start=(j == 0), stop=False,
                                      skip_group_check=True)
                    nc_.tensor.matmul(pp[:, j, :], ones1, brow_t,
                                      start=False, stop=(j == 3),
                                      skip_group_check=True)
                ot = outp.tile([128, 4, 128], F32, tag="ot", name=f"ot{w}")
                nc_.scalar.activation(out=ot, in_=pp, func=COPY)
                yap = y[b, n0:n0 + W, :].rearrange("(j p) c -> p j c", p=128)
                nc_.sync.dma_start(out=yap, in_=ot)

            def emit_post(w):
                # snapshot cumulative z, diff vs two windows back, normalize,
                # and DMA-transpose [n, (h d)] -> attT[b]
                b, nb = divmod(w, NB)
                n0 = nb * W
                av = avs.pop(w)
                snap = nrm.tile([128, 2, 4, 1], F32, tag="snap",
                                name=f"snap{w}")
                nc_.vector.tensor_copy(out=snap, in_=zzs[:, w % 2])
                snaps[w] = snap
                zdiff = nrm.tile([128, 2, 4, 1], F32, tag="zdiff")
                nc_.vector.tensor_tensor(out=zdiff, in0=snap,
                                         in1=snaps.pop(w - 2),
                                         op=mybir.AluOpType.subtract)
                zinv = nrm.tile([128, 2, 4, 1], F32, tag="zinv")
                nc_.vector.reciprocal_approx_fast(
                    out=zinv.rearrange("p h j o -> p (h j o)"),
                    in_=zdiff.rearrange("p h j o -> p (h j o)"))
                attn_n = atn.tile([128, 4, 2, 64], F16, tag="attn",
                                  name=f"attn{w}")
                nc_.vector.tensor_tensor(
                    out=attn_n.rearrange("p j h d -> p h j d"),
                    in0=av,
                    in1=zinv.broadcast_to([128, 2, 4, 64]),
                    op=mybir.AluOpType.mult,
                )
                for j in range(4):
                    nc_.sync.dma_start_transpose(
                        out=attT[b][:, n0 + j * 128:n0 + (j + 1) * 128],
                        in_=attn_n[:, j, :, :],
                    )

            for k in range(len(seq) + LAG):
                if k < len(seq):
                    b, nb, mt = seq[k]
                    w = k // MT
                    if mt == 0:
                        avs[w] = avp.tile([128, 2, 4, 64], F32, tag="av",
                                          name=f"av{w}")
                    n0 = nb * W
                    m0 = mt * 128
                    st = stp.tile([128, 2, 512], F32, tag="st")
                    for h in (1, 0):
                        hs = slice(h * 64, (h + 1) * 64)
                        nc_.tensor.matmul(
                            st[:, h, :],
                            _r(kT[b][hs, m0:m0 + 128]),
                            _r(qT[b][hs, n0:n0 + W]),
                            start=True, stop=True,
                        )
                    pt = ptp.tile([128, 2, 4, 128], F16, tag="pt")
                    if mt in ALL_ACT_MTS:
                        nc_.scalar.activation(out=pt, in_=st, func=EXP,
                                              scale=SCALE, bias=ebias)
                    else:
                        nc_.vector.tensor_scalar(
                            out=pt.bitcast(I16)[:, 1],
                            in0=st[:, 1],
                            scalar1=SCL1,
                            scalar2=SCL2,
                            op0=mybir.AluOpType.mult,
                            op1=mybir.AluOpType.add,
                        )
                        nc_.scalar.activation(out=pt[:, 0], in_=st[:, 0],
                                              func=EXP, scale=SCALE,
                                              bias=ebias)
                    pts[k] = pt

                kk = k - LAG
                if kk < 0:
                    continue
                b2, nb2, mt2 = seq[kk]
                w2 = kk // MT
                av = avs[w2]
                pt2 = pts.pop(kk)
                for h in range(2):
                    for j in range(4):
                        nc_.tensor.matmul(
                            av[:, h, j, :],
                            pt2[:, h, j, :],
                            vaug[b2][:, mt2, h, :],
                            start=(mt2 == 0 and h == 0 and j == 0),
                            stop=(mt2 == MT - 1),
                            skip_group_check=True,
                        )
                        nc_.tensor.matmul(
                            zzs[:, w2 % 2, h, j, :],
                            pt2[:, h, j, :],
                            onesm,
                            start=(w2 == 0 and mt2 == 0 and h == 0 and j == 0),
                            stop=(mt2 == MT - 1),
                            skip_group_check=True,
                        )
                if mt2 == 2 and w2 > 0:
                    emit_post(w2 - 1)
                if mt2 == 8 and w2 > 0:
                    emit_proj(w2 - 1)
            emit_post(NW - 1)
            emit_proj(NW - 1)

    nc.finalize()
    return nc


def _core_inputs(x, w_qkv, w_proj, b_proj, c):
    h0 = 2 * c
    gq, oq = divmod(64 * h0, 384)
    gk, ok = divmod(C + 64 * h0, 384)
    gv, ov = divmod(2 * C + 64 * h0, 384)

    def xsl(g, dt=np.float32):
        # [B,N,128] slice -> channel-major [128, B*N]
        return np.ascontiguousarray(
            x[:, :, 128 * g:128 * (g + 1)].reshape(BN, 128).T
        ).astype(dt)

    return {
        "xq": xsl(gq),
        "xk": xsl(gk),
        "xv": xsl(gv, np.float16),
        "wq": np.ascontiguousarray(w_qkv[gq][:, oq:oq + 128]),
        "wk": np.ascontiguousarray(w_qkv[gk][:, ok:ok + 128]),
        "wv": np.ascontiguousarray(w_qkv[gv][:, ov:ov + 128]).astype(np.float16),
        "wp": np.ascontiguousarray(w_proj[c]).astype(np.float16),
        "brow": b_proj[128 * c:128 * (c + 1)].reshape(1, 128).astype(np.float16),
    }


def kernel(x, w_qkv, w_proj, b_proj, _trace=False, _trace_kwargs=None):
    x = np.asarray(x, np.float32)
    w_qkv = np.asarray(w_qkv, np.float32)
    w_proj = np.asarray(w_proj, np.float32)
    b_proj = np.asarray(b_proj, np.float32)

    if "nc" not in _CACHE:
        _CACHE["nc"] = _build_nc()
    nc = _CACHE["nc"]

    in_maps = [_core_inputs(x, w_qkv, w_proj, b_proj, c) for c in range(8)]
    res = run_bass_kernel_spmd(
        nc, in_maps, list(range(8)),
        trace=_trace, **(_trace_kwargs or {}),
    )
    out = np.concatenate([res.results[c]["y"] for c in range(8)], axis=2)
    if _trace:
        return out, res
    return out


# revision 18
# speedup vs baseline: 1.0218x; 1.0218x over previous
"""GroupedAttention Trainium2 kernel (8 NeuronCores, SPMD, no collectives).

Problem: x[2,2048,1024] -> grouped qkv (G=8 block-diag) -> 16-head attention
-> grouped proj (G=8 block-diag) + bias.

Sharding: core c owns heads (2c, 2c+1) and proj group c. The proj group c
consumes exactly the attention outputs of heads 2c/2c+1 and produces output
channels [128c, 128c+128) -- each core computes an independent channel
slice of the final output; outputs are concatenated on the host.

Engine plan (per core):
- PE: q/k/v projections (f32r / fp16), scores S=K^T.Q (f32r, 512-col moving),
  AV with stationary P-tile [m, n-block] and moving V|ones [m, 65] fp16
  accumulating out[n, 65] over m-tiles (the 65th column collects the softmax
  denominator), proj (fp16) with bias folded in via a K=1 ones-row matmul.
- ACT: part of the exp tiles (table Exp, scale folded in), psum->sbuf copies.
- DVE: the other exp tiles via Schraudolph bit-trick exp
  (fp16 = bitcast(int16(x*SCALE*log2e*1024 + 15315.26))), v copies,
  per-partition softmax normalization.
- SP (sync): all DMA issue - input loads, XBAR dma-transposes of the
  normalized attention tiles [n,128]->[128,n], output stores.
"""

import numpy as np
from contextlib import ExitStack

import concourse.bass as bass
import concourse.tile as tile
from concourse import bacc, mybir
from concourse.bass_utils import run_bass_kernel_spmd

F32 = mybir.dt.float32
F32R = mybir.dt.float32r
F16 = mybir.dt.float16
I16 = mybir.dt.int16
EXP = mybir.ActivationFunctionType.Exp
COPY = mybir.ActivationFunctionType.Copy

B = 2
N = 2048
C = 1024
H = 16
G = 8
D = 64          # head dim
BN = B * N      # 4096
W = 512         # attention n-window per round
NB = N // W     # rounds per batch = 4
MT = N // 128   # m-tiles per batch = 16
SCALE = D ** -0.5
LOG2E = 1.4426950408889634
# Schraudolph fp16 exp: fp16 = bitcast(int16(x*SCL1 + SCL2))
SCL1 = SCALE * LOG2E * 1024.0
SCL2 = 15315.26
# of the 16 m-tiles per window, this many run exp on ACT (rest on DVE)
ACT_TILES = 16

_CACHE = {}


def _r(ap):
    return ap if ap.dtype == F32R else ap.bitcast(F32R)


def _build_nc():
    nc = bacc.Bacc("TRN2", target_bir_lowering=False, debug=False, num_devices=8)

    xq = nc.dram_tensor("xq", [128, BN], F32, kind="ExternalInput").ap()
    xk = nc.dram_tensor("xk", [128, BN], F32, kind="ExternalInput").ap()
    xv = nc.dram_tensor("xv", [128, BN], F16, kind="ExternalInput").ap()
    wq = nc.dram_tensor("wq", [128, 128], F32, kind="ExternalInput").ap()
    wk = nc.dram_tensor("wk", [128, 128], F32, kind="ExternalInput").ap()
    wv = nc.dram_tensor("wv", [128, 128], F16, kind="ExternalInput").ap()
    wp = nc.dram_tensor("wp", [128, 128], F16, kind="ExternalInput").ap()
    brow = nc.dram_tensor("brow", [1, 128], F16, kind="ExternalInput").ap()
    y = nc.dram_tensor("y", [B, N, 128], F32, kind="ExternalOutput").ap()

    with ExitStack() as ctx:
        tc = ctx.enter_context(tile.TileContext(nc))
        nc_ = tc.nc

        persist = ctx.enter_context(tc.tile_pool(name="persist", bufs=1))

        # ---- weights / constants ----
        wk_t = persist.tile([128, 128], F32R, tag="wk")
        nc_.gpsimd.dma_start(out=wk_t, in_=wk)
        wq_t = persist.tile([128, 128], F32R, tag="wq")
        nc_.gpsimd.dma_start(out=wq_t, in_=wq)
        wv_t = persist.tile([128, 128], F16, tag="wv")
        nc_.sync.dma_start(out=wv_t, in_=wv)
        wp_t = persist.tile([128, 128], F16, tag="wp")
        nc_.sync.dma_start(out=wp_t, in_=wp)
        brow_t = persist.tile([1, 128], F16, tag="brow")
        nc_.sync.dma_start(out=brow_t, in_=brow)
        ones1 = persist.tile([1, 128], F16, tag="ones1")
        nc_.gpsimd.memset(ones1, 1.0)
        onesm = persist.tile([128, 1], F16, tag="onesm")
        nc_.gpsimd.memset(onesm, 1.0)
        ebias = persist.tile([128, 1], F32, tag="ebias")
        nc_.gpsimd.memset(ebias, EXP_BIAS)
        zero8 = persist.tile([128, 2, 4, 1], F32, tag="zero8")
        nc_.gpsimd.memset(zero8, 0.0)


        # ---- x slices (channel-major) ----
        xq_t = persist.tile([128, BN], F32R, tag="xq")
        xk_t = persist.tile([128, BN], F32R, tag="xk")
        xv_t = persist.tile([128, BN], F16, tag="xv")

        # persistent activations
        qT = [persist.tile([128, N], F32R, tag=f"qT{b}", name=f"qT{b}")
              for b in range(B)]   # rows 0:64 h0, 64:128 h1 (pre-scaled? no)
        kT = [persist.tile([128, N], F32R, tag=f"kT{b}", name=f"kT{b}")
              for b in range(B)]
        # vaug[b]: [128(m), MT, 2(h), 64] fp16
        vaug = [persist.tile([128, MT, 2, 64], F16, tag=f"vaug{b}", name=f"vaug{b}")
                for b in range(B)]
        # transposed, normalized attention outputs per b: [128(2h*d), N] fp16
        attT = [persist.tile([128, N], F16, tag=f"attT{b}", name=f"attT{b}")
                for b in range(B)]

        # ---- load x: per-batch chunks ordered k, v, q so windows start early
        for b in range(B):
            s = slice(b * N, (b + 1) * N)
            for i in range(2):
                ss = slice(b * N + i * 1024, b * N + (i + 1) * 1024)
                nc_.gpsimd.dma_start(out=xk_t[:, ss], in_=xk[:, ss])
            nc_.sync.dma_start(out=xv_t[:, s], in_=xv[:, s])
            for i in range(2):
                ss = slice(b * N + i * 1024, b * N + (i + 1) * 1024)
                nc_.gpsimd.dma_start(out=xq_t[:, ss], in_=xq[:, ss])

        # ---- phase 1: kT / vaug / qT per batch ----
        with tc.tile_pool(name="ph1", bufs=3, space="PSUM") as ph1:
            for b in range(B):
                for i in range(4):
                    sl = slice(i * 512, (i + 1) * 512)
                    s = slice(b * N + i * 512, b * N + (i + 1) * 512)
                    pk = ph1.tile([128, 512], F32, tag="qk")
                    nc_.tensor.matmul(pk, _r(wk_t), _r(xk_t[:, s]), start=True, stop=True)
                    nc_.scalar.activation(out=kT[b][:, sl], in_=pk, func=COPY)
                for mt in range(MT):
                    g = b * MT + mt
                    pv = ph1.tile([128, 128], F32, tag="v")
                    nc_.tensor.matmul(
                        pv, xv_t[:, g * 128:(g + 1) * 128], wv_t,
                        start=True, stop=True,
                    )
                    # both heads' v [128, (2,64)] -> vaug[b][:, mt]
                    nc_.vector.tensor_copy(out=vaug[b][:, mt], in_=pv)
                for i in range(4):
                    sl = slice(i * 512, (i + 1) * 512)
                    s = slice(b * N + i * 512, b * N + (i + 1) * 512)
                    pq = ph1.tile([128, 512], F32, tag="qk")
                    nc_.tensor.matmul(pq, _r(wq_t), _r(xq_t[:, s]), start=True, stop=True)
                    nc_.scalar.activation(out=qT[b][:, sl], in_=pq, func=COPY)

        # ---- phase 2: flat software-pipelined attention + proj ----
        # Iterations k = (window, m-tile); scores(k) + exp(k) are emitted at
        # k, AV(k - LAG) trails. Z rides in a single never-reset cumulative
        # PSUM bank indexed by window parity (Z_w = cum_w - cum_{w-2}), so
        # nothing has to read window state right at the window boundary:
        # normalization for window w is deferred into window w+1.
        with tc.tile_pool(name="stp", bufs=2, space="PSUM") as stp, \
             tc.tile_pool(name="avp", bufs=2, space="PSUM") as avp, \
             tc.tile_pool(name="prj", bufs=1, space="PSUM") as prj, \
             tc.tile_pool(name="ptp", bufs=5) as ptp, \
             tc.tile_pool(name="nrm", bufs=3) as nrm, \
             tc.tile_pool(name="atn", bufs=2) as atn, \
             tc.tile_pool(name="outp", bufs=2) as outp:

            LAG = 3
            NW = B * NB
            seq = [(b, nb, mt) for b in range(B) for nb in range(NB)
                   for mt in range(MT)]
            zzs = avp.tile([128, 2, 2, 4, 1], F32, tag="zzs", bufs=1,
                           name="zzs")
            avs = {}
            pts = {}
            snaps = {-2: zero8, -1: zero8}

            def emit_proj(w):
                # out[n, cout] = attT^T @ wp + ones^T @ bias
                b, nb = divmod(w, NB)
                n0 = nb * W
                pp = prj.tile([128, 4, 128], F32, tag="pp", name=f"pp{w}")
                for j in range(4):
                    nt = slice(n0 + j * 128, n0 + (j + 1) * 128)
                    nc_.tensor.matmul(pp[:, j, :], attT[b][:, nt], wp_t,
                                      start=(j == 0), stop=False,
                                      skip_group_check=True)
                    nc_.tensor.matmul(pp[:, j, :], ones1, brow_t,
                                      start=False, stop=(j == 3),
                                      skip_group_check=True)
                ot = outp.tile([128, 4, 128], F32, tag="ot", name=f"ot{w}")
                nc_.scalar.activation(out=ot, in_=pp, func=COPY)
                yap = y[b, n0:n0 + W, :].rearrange("(j p) c -> p j c", p=128)
                nc_.sync.dma_start(out=yap, in_=ot)

            def emit_post(w):
                # snapshot cumulative z, diff vs two windows back, normalize,
                # and DMA-transpose [n, (h d)] -> attT[b]
                b, nb = divmod(w, NB)
                n0 = nb * W
                av = avs.pop(w)
                snap = nrm.tile([128, 2, 4, 1], F32, tag="snap",
                                name=f"snap{w}")
                nc_.vector.tensor_copy(out=snap, in_=zzs[:, w % 2])
                snaps[w] = snap
                zdiff = nrm.tile([128, 2, 4, 1], F32, tag="zdiff")
                nc_.vector.tensor_tensor(out=zdiff, in0=snap,
                                         in1=snaps.pop(w - 2),
                                         op=mybir.AluOpType.subtract)
                zinv = nrm.tile([128, 2, 4, 1], F32, tag="zinv")
                nc_.vector.reciprocal_approx_fast(
                    out=zinv.rearrange("p h j o -> p (h j o)"),
                    in_=zdiff.rearrange("p h j o -> p (h j o)"))
                attn_n = atn.tile([128, 4, 2, 64], F16, tag="attn",
                                  name=f"attn{w}")
                nc_.vector.tensor_tensor(
                    out=attn_n.rearrange("p j h d -> p h j d"),
                    in0=av,
                    in1=zinv.broadcast_to([128, 2, 4, 64]),
                    op=mybir.AluOpType.mult,
                )
                for j in range(4):
                    nc_.sync.dma_start_transpose(
                        out=attT[b][:, n0 + j * 128:n0 + (j + 1) * 128],
                        in_=attn_n[:, j, :, :],
                    )

            for k in range(len(seq) + LAG):
                if k < len(seq):
                    b, nb, mt = seq[k]
                    w = k // MT
                    if mt == 0:
                        avs[w] = avp.tile([128, 2, 4, 64], F32, tag="av",
                                          name=f"av{w}")
                    n0 = nb * W
                    m0 = mt * 128
                    st = stp.tile([128, 2, 512], F32, tag="st")
                    for h in (1, 0):
                        hs = slice(h * 64, (h + 1) * 64)
                        nc_.tensor.matmul(
                            st[:, h, :],
                            _r(kT[b][hs, m0:m0 + 128]),
                            _r(qT[b][hs, n0:n0 + W]),
                            start=True, stop=True,
                        )
                    pta = ptp.tile([128, 2, 4, 128], F16, tag="pta")
                    ptd = ptp.tile([128, 4, 128], I16, tag="ptd")
                    if mt in ALL_ACT_MTS:
                        nc_.scalar.activation(out=pta, in_=st, func=EXP,
                                              scale=SCALE, bias=ebias)
                        pts[k] = (pta, pta[:, 1])
                    else:
                        nc_.vector.tensor_scalar(
                            out=ptd,
                            in0=st[:, 1],
                            scalar1=SCL1,
                            scalar2=SCL2,
                            op0=mybir.AluOpType.mult,
                            op1=mybir.AluOpType.add,
                        )
                        nc_.scalar.activation(out=pta[:, 0], in_=st[:, 0],
                                              func=EXP, scale=SCALE,
                                              bias=ebias)
                        pts[k] = (pta, ptd.bitcast(F16))

                kk = k - LAG
                if kk < 0:
                    continue
                b2, nb2, mt2 = seq[kk]
                w2 = kk // MT
                av = avs[w2]
                pta2, pth1 = pts.pop(kk)
                for h in range(2):
                    lhs_j = (lambda j: pta2[:, 0, j, :]) if h == 0 else \
                        (lambda j: pth1[:, j, :])
                    for j in range(4):
                        nc_.tensor.matmul(
                            av[:, h, j, :],
                            lhs_j(j),
                            vaug[b2][:, mt2, h, :],
                            start=(mt2 == 0 and h == 0 and j == 0),
                            stop=(mt2 == MT - 1),
                            skip_group_check=True,
                        )
                        nc_.tensor.matmul(
                            zzs[:, w2 % 2, h, j, :],
                            lhs_j(j),
                            onesm,
                            start=(w2 == 0 and mt2 == 0 and h == 0 and j == 0),
                            stop=(mt2 == MT - 1),
                            skip_group_check=True,
                        )
                if mt2 == 2 and w2 > 0:
                    emit_post(w2 - 1)
                if mt2 == 8 and w2 > 0:
                    emit_proj(w2 - 1)
            emit_post(NW - 1)
            emit_proj(NW - 1)

    nc.finalize()
    return nc


def _core_inputs(x, w_qkv, w_proj, b_proj, c):
    h0 = 2 * c
    gq, oq = divmod(64 * h0, 384)
    gk, ok = divmod(C + 64 * h0, 384)
    gv, ov = divmod(2 * C + 64 * h0, 384)

    def xsl(g, dt=np.float32):
        # [B,N,128] slice -> channel-major [128, B*N]
        return np.ascontiguousarray(
            x[:, :, 128 * g:128 * (g + 1)].reshape(BN, 128).T
        ).astype(dt)

    return {
        "xq": xsl(gq),
        "xk": xsl(gk),
        "xv": xsl(gv, np.float16),
        "wq": np.ascontiguousarray(w_qkv[gq][:, oq:oq + 128]),
        "wk": np.ascontiguousarray(w_qkv[gk][:, ok:ok + 128]),
        "wv": np.ascontiguousarray(w_qkv[gv][:, ov:ov + 128]).astype(np.float16),
        "wp": np.ascontiguousarray(w_proj[c]).astype(np.float16),
        "brow": b_proj[128 * c:128 * (c + 1)].reshape(1, 128).astype(np.float16),
    }


def kernel(x, w_qkv, w_proj, b_proj, _trace=False, _trace_kwargs=None):
    x = np.asarray(x, np.float32)
    w_qkv = np.asarray(w_qkv, np.float32)
    w_proj = np.asarray(w_proj, np.float32)
    b_proj = np.asarray(b_proj, np.float32)

    if "nc" not in _CACHE:
        _CACHE["nc"] = _build_nc()
    nc = _CACHE["nc"]

    in_maps = [_core_inputs(x, w_qkv, w_proj, b_proj, c) for c in range(8)]
    res = run_bass_kernel_spmd(
        nc, in_maps, list(range(8)),
        trace=_trace, **(_trace_kwargs or {}),
    )
    out = np.concatenate([res.results[c]["y"] for c in range(8)], axis=2)
    if _trace:
        return out, res
    return out
